# revision 16
# baseline (speedup 1.0000x reference)
"""HSE (hard squeeze-excite) Trainium2 Bass kernel.

Full inputs: x [32,56,56,256] f32, w1 [256,64], w2 [64,256].
out = x * hsigmoid(relu6(gap(x) @ w1) @ w2), gap = mean over H,W.

Sharding: pure data-parallel over batch, 4 samples per core on 8 cores.

Per-core layout (pair-granule pipeline): 3136 = 64*49, so one PAIR of
samples fills all 128 partitions: granule m holds sample 2m on
partitions 0-63 and sample 2m+1 on partitions 64-127, each partition
line holding 49 contiguous tokens. The two granules form a software
pipeline: while granule 1 streams in, granule 0 runs its squeeze/excite
chain, gate multiply and streams out. All big DMAs sit on ONE HWDGE
ring (sync engine) in program order, so the HBM stream never idles:
loads0, loads1, stores0, stores1 back to back at line rate.
"""

import numpy as np

B, H, W, C = 32, 56, 56, 256
CR = 64
NCORES = 8
BPC = B // NCORES            # 4 samples per core
TOK = H * W                  # 3136 tokens per sample
P = 128                      # SBUF partitions
NG = BPC // 2                # 2 granules (sample pairs) per core
TPL = TOK // 64              # 49 tokens per partition line
HP = 64                      # partitions per sample within a granule

_CACHE = {}


def _tree_sum(nc, op, out, xc, tmp, n):
    """Sum n token slices xc[:, 0:n, :] into out [P, C] with contiguous
    pairwise adds on the vector engine. tmp needs n//2 slots."""
    rem = []
    h = n // 2
    if n % 2:
        rem.append(xc[:, n - 1, :])
    if h >= 2:
        nc.vector.tensor_tensor(tmp[:, 0:h, :], xc[:, 0:h, :], xc[:, h : 2 * h, :], op=op.add)
    else:
        dst = tmp[:, 0, :] if rem else out
        nc.vector.tensor_tensor(dst, xc[:, 0, :], xc[:, 1, :], op=op.add)
    while h > 1:
        if h % 2:
            rem.append(tmp[:, h - 1, :])
        h //= 2
        if h >= 2:
            nc.vector.tensor_tensor(tmp[:, 0:h, :], tmp[:, 0:h, :], tmp[:, h : 2 * h, :], op=op.add)
        else:
            dst = tmp[:, 0, :] if rem else out
            nc.vector.tensor_tensor(dst, tmp[:, 0, :], tmp[:, 1, :], op=op.add)
    cur = tmp[:, 0, :]
    for i, r in enumerate(rem):
        dst = out if i == len(rem) - 1 else tmp[:, 0, :]
        nc.vector.tensor_tensor(dst, cur, r, op=op.add)
        cur = tmp[:, 0, :]


def _build():
    import concourse.bacc as bacc
    import concourse.tile as tile
    import concourse.mybir as mybir

    f32 = mybir.dt.float32
    op = mybir.AluOpType

    nc = bacc.Bacc("TRN2", target_bir_lowering=False, debug=False)

    # x viewed per granule: [granule, half, 64 lines, 49 tokens, 256]
    x_d = nc.dram_tensor("x", [NG, 2, HP, TPL, C], f32, kind="ExternalInput").ap()
    w1_d = nc.dram_tensor("w1", [C, CR], f32, kind="ExternalInput").ap()
    w2_d = nc.dram_tensor("w2", [CR, C], f32, kind="ExternalInput").ap()
    mask_d = nc.dram_tensor("mask2", [P, 2], f32, kind="ExternalInput").ap()
    rt_d = nc.dram_tensor("rt2", [2, P], f32, kind="ExternalInput").ap()
    o_d = nc.dram_tensor("out", [NG, 2, HP, TPL, C], f32, kind="ExternalOutput").ap()

    # token chunks within a granule (pipeline grain for DMA + tree + mult)
    CHUNKS = [(0, 16), (16, 32), (32, 49)]

    with tile.TileContext(nc) as tc:
        with tc.tile_pool(name="big", bufs=1) as big, \
             tc.tile_pool(name="small", bufs=1) as small, \
             tc.tile_pool(name="gpool", bufs=2) as gpool, \
             tc.tile_pool(name="psum", bufs=1, space="PSUM") as psum, \
             tc.tile_pool(name="gps", bufs=2, space="PSUM") as gps:

            X = big.tile([P, NG, TPL, C], f32)      # both granules, ~100KB/part
            tmp = small.tile([P, 8, C], f32)        # tree scratch
            gpart = small.tile([P, 3, C], f32)      # per-chunk token sums
            partial = small.tile([P, C], f32)       # per-line token sums
            w1s = small.tile([P, 2, CR], f32)
            w2s = small.tile([CR, C], f32)
            mask2 = small.tile([P, 2], f32)         # line -> sample-in-pair (1/TOK)
            rt2 = small.tile([2, P], f32)           # sample-in-pair -> lines

            # weights + constants on the scalar ring; sync ring is the
            # ordered bulk stream
            nc.scalar.dma_start(w1s[:, 0, :], w1_d[0:P, :])
            nc.scalar.dma_start(w1s[:, 1, :], w1_d[P : 2 * P, :])
            nc.scalar.dma_start(w2s[:], w2_d[:])
            nc.scalar.dma_start(mask2[:], mask_d[:])
            nc.scalar.dma_start(rt2[:], rt_d[:])

            # ---- loads: granule 0 then granule 1, chunked, one ring ----
            for m in range(NG):
                for (t0, t1) in CHUNKS:
                    nc.sync.dma_start(
                        X[0:HP, m, t0:t1, :], x_d[m, 0, :, t0:t1, :]
                    )
                    nc.sync.dma_start(
                        X[HP:P, m, t0:t1, :], x_d[m, 1, :, t0:t1, :]
                    )

            G_sb = [None] * NG
            for m in range(NG):
                # ---- GAP: per-chunk contiguous tree adds ----
                for ci, (t0, t1) in enumerate(CHUNKS):
                    _tree_sum(nc, op, gpart[:, ci, :], X[:, m, t0:t1, :], tmp, t1 - t0)
                nc.vector.tensor_tensor(tmp[:, 0, :], gpart[:, 0, :], gpart[:, 1, :], op=op.add)
                nc.vector.tensor_tensor(partial[:], tmp[:, 0, :], gpart[:, 2, :], op=op.add)

                # ---- squeeze/excite chain for this pair ----
                # sT[c, j] = sum_p partial[p, c] * mask2[p, j]  (mean, transposed)
                sT_ps = psum.tile([P, 4], f32, tag="sT")
                nc.tensor.matmul(sT_ps[:, 0:2], partial[:, 0:P], mask2[:], start=True, stop=True)
                nc.tensor.matmul(sT_ps[:, 2:4], partial[:, P : 2 * P], mask2[:], start=True, stop=True)
                sT_sb = small.tile([P, 4], f32, tag="sTsb")
                nc.scalar.copy(sT_sb[:], sT_ps[:])

                # zT[r, j] = sum_c w1[c, r] * sT[c, j]
                zT_ps = psum.tile([CR, 2], f32, tag="zT")
                nc.tensor.matmul(zT_ps[:], w1s[:, 0, :], sT_sb[:, 0:2], start=True, stop=False)
                nc.tensor.matmul(zT_ps[:], w1s[:, 1, :], sT_sb[:, 2:4], start=False, stop=True)
                zT_sb = small.tile([CR, 2], f32, tag="zTsb")
                nc.vector.tensor_scalar(zT_sb[:], zT_ps[:], 0.0, 6.0, op0=op.max, op1=op.min)

                # y[j, c] = sum_r zT[r, j] * w2[r, c]
                y_ps = psum.tile([2, C], f32, tag="y")
                nc.tensor.matmul(y_ps[:], zT_sb[:], w2s[:], start=True, stop=True)
                g_sb = small.tile([2, C], f32, tag="g")
                nc.vector.tensor_scalar(g_sb[:], y_ps[:], 3.0, 0.0, op0=op.add, op1=op.max)
                nc.vector.tensor_scalar(g_sb[:], g_sb[:], 6.0, 1.0 / 6.0, op0=op.min, op1=op.mult)

                # replicate gate rows onto lines: G[p, c] = g[p // HP, c]
                G_ps = gps.tile([P, C], f32, tag="G")
                nc.tensor.matmul(G_ps[:], rt2[:], g_sb[:], start=True, stop=True)
                G_sb[m] = gpool.tile([P, C], f32, tag="Gsb", name=f"G_sb{m}")
                nc.scalar.copy(G_sb[m][:], G_ps[:])

                # ---- gate multiply in place + stores on the same ring ----
                for (t0, t1) in CHUNKS:
                    xc = X[:, m, t0:t1, :]
                    gb = G_sb[m][:].unsqueeze(1).broadcast_to([P, t1 - t0, C])
                    nc.vector.tensor_tensor(xc, xc, gb, op=op.mult)
                    nc.sync.dma_start(o_d[m, 0, :, t0:t1, :], X[0:HP, m, t0:t1, :])
                    nc.sync.dma_start(o_d[m, 1, :, t0:t1, :], X[HP:P, m, t0:t1, :])

    nc.compile()
    return nc


def _in_maps(x, w1, w2):
    x = np.ascontiguousarray(x, dtype=np.float32)
    w1 = np.ascontiguousarray(w1, dtype=np.float32)
    w2 = np.ascontiguousarray(w2, dtype=np.float32)

    mask2 = np.zeros((P, 2), dtype=np.float32)
    rt2 = np.zeros((2, P), dtype=np.float32)
    for j in range(2):
        mask2[HP * j : HP * (j + 1), j] = 1.0 / TOK
        rt2[j, HP * j : HP * (j + 1)] = 1.0

    in_maps = []
    for c in range(NCORES):
        # [4 samples, 3136 tok, C] -> [NG, 2, HP, TPL, C]
        shard = x[c * BPC : (c + 1) * BPC].reshape(NG, 2, HP, TPL, C)
        in_maps.append({"x": shard, "w1": w1, "w2": w2, "mask2": mask2, "rt2": rt2})
    return in_maps


def kernel(x, w1, w2):
    from concourse.bass_utils import run_bass_kernel_spmd

    if "nc" not in _CACHE:
        _CACHE["nc"] = _build()
    nc = _CACHE["nc"]

    res = run_bass_kernel_spmd(nc, _in_maps(x, w1, w2), core_ids=list(range(NCORES)))
    out = np.empty((B, H, W, C), dtype=np.float32)
    for c in range(NCORES):
        out[c * BPC : (c + 1) * BPC] = res.results[c]["out"].reshape(BPC, H, W, C)
    return out


# revision 18
# speedup vs baseline: 1.2560x; 1.2560x over previous
"""HSE (hard squeeze-excite) Trainium2 Bass kernel.

Full inputs: x [32,56,56,256] f32, w1 [256,64], w2 [64,256].
out = x * hsigmoid(relu6(gap(x) @ w1) @ w2), gap = mean over H,W.

Sharding: pure data-parallel over batch, 4 samples per core on 8 cores.

Per-core layout (pair-granule pipeline): 3136 = 64*49, so one PAIR of
samples fills all 128 partitions: granule m holds sample 2m on
partitions 0-63 and sample 2m+1 on partitions 64-127, each partition
line holding 49 contiguous tokens. The two granules form a software
pipeline: while granule 1 streams in, granule 0 runs its squeeze/excite
chain, gate multiply and streams out. All big DMAs sit on ONE HWDGE
ring (sync engine) in program order, so the HBM stream never idles:
loads0, loads1, stores0, stores1 back to back at line rate.
"""

import numpy as np

B, H, W, C = 32, 56, 56, 256
CR = 64
NCORES = 8
BPC = B // NCORES            # 4 samples per core
TOK = H * W                  # 3136 tokens per sample
P = 128                      # SBUF partitions
NG = BPC // 2                # 2 granules (sample pairs) per core
TPL = TOK // 64              # 49 tokens per partition line
HP = 64                      # partitions per sample within a granule

_CACHE = {}


def _tree_sum(nc, op, out, xc, tmp, n):
    """Sum n token slices xc[:, 0:n, :] into out [P, C] with contiguous
    pairwise adds on the vector engine. tmp needs n//2 slots."""
    rem = []
    h = n // 2
    if n % 2:
        rem.append(xc[:, n - 1, :])
    if h >= 2:
        nc.vector.tensor_tensor(tmp[:, 0:h, :], xc[:, 0:h, :], xc[:, h : 2 * h, :], op=op.add)
    else:
        dst = tmp[:, 0, :] if rem else out
        nc.vector.tensor_tensor(dst, xc[:, 0, :], xc[:, 1, :], op=op.add)
    while h > 1:
        if h % 2:
            rem.append(tmp[:, h - 1, :])
        h //= 2
        if h >= 2:
            nc.vector.tensor_tensor(tmp[:, 0:h, :], tmp[:, 0:h, :], tmp[:, h : 2 * h, :], op=op.add)
        else:
            dst = tmp[:, 0, :] if rem else out
            nc.vector.tensor_tensor(dst, tmp[:, 0, :], tmp[:, 1, :], op=op.add)
    cur = tmp[:, 0, :]
    for i, r in enumerate(rem):
        dst = out if i == len(rem) - 1 else tmp[:, 0, :]
        nc.vector.tensor_tensor(dst, cur, r, op=op.add)
        cur = tmp[:, 0, :]


def _build():
    import concourse.bacc as bacc
    import concourse.tile as tile
    import concourse.mybir as mybir

    f32 = mybir.dt.float32
    op = mybir.AluOpType

    nc = bacc.Bacc("TRN2", target_bir_lowering=False, debug=False)

    # x viewed per granule: [granule, half, 64 lines, 49 tokens, 256]
    x_d = nc.dram_tensor("x", [NG, 2, HP, TPL, C], f32, kind="ExternalInput").ap()
    w1_d = nc.dram_tensor("w1", [C, CR], f32, kind="ExternalInput").ap()
    w2_d = nc.dram_tensor("w2", [CR, C], f32, kind="ExternalInput").ap()
    mask_d = nc.dram_tensor("mask2", [P, 2], f32, kind="ExternalInput").ap()
    rt_d = nc.dram_tensor("rt2", [2, P], f32, kind="ExternalInput").ap()
    o_d = nc.dram_tensor("out", [NG, 2, HP, TPL, C], f32, kind="ExternalOutput").ap()

    # token chunks within a granule (pipeline grain for DMA + tree + mult)
    CHUNKS = [(0, 16), (16, 32), (32, 49)]

    with tile.TileContext(nc) as tc:
        with tc.tile_pool(name="big", bufs=1) as big, \
             tc.tile_pool(name="small", bufs=1) as small, \
             tc.tile_pool(name="gpool", bufs=2) as gpool, \
             tc.tile_pool(name="psum", bufs=1, space="PSUM") as psum, \
             tc.tile_pool(name="gps", bufs=2, space="PSUM") as gps:

            X = big.tile([P, NG, TPL, C], f32)      # both granules, ~100KB/part
            tmp = small.tile([P, 8, C], f32)        # tree scratch
            gpart = small.tile([P, 3, C], f32)      # per-chunk token sums
            partial = small.tile([P, C], f32)       # per-line token sums
            w1s = small.tile([P, 2, CR], f32)
            w2s = small.tile([CR, C], f32)
            mask2 = small.tile([P, 2], f32)         # line -> sample-in-pair (1/TOK)
            rt2 = small.tile([2, P], f32)           # sample-in-pair -> lines

            # weights + constants on the scalar ring; sync ring is the
            # ordered bulk stream
            nc.scalar.dma_start(w1s[:, 0, :], w1_d[0:P, :])
            nc.scalar.dma_start(w1s[:, 1, :], w1_d[P : 2 * P, :])
            nc.scalar.dma_start(w2s[:], w2_d[:])
            nc.scalar.dma_start(mask2[:], mask_d[:])
            nc.scalar.dma_start(rt2[:], rt_d[:])

            # ---- loads: granule 0 then granule 1, chunked, one ring ----
            for m in range(NG):
                for (t0, t1) in CHUNKS:
                    # one 128-partition DMA per chunk: DRAM side [2,64,t,C]
                    # iterates in the same element order as SBUF [128,t,C]
                    nc.sync.dma_start(
                        X[:, m, t0:t1, :], x_d[m, :, :, t0:t1, :]
                    )

            G_sb = [None] * NG
            for m in range(NG):
                # ---- GAP: per-chunk contiguous tree adds ----
                for ci, (t0, t1) in enumerate(CHUNKS):
                    _tree_sum(nc, op, gpart[:, ci, :], X[:, m, t0:t1, :], tmp, t1 - t0)
                nc.vector.tensor_tensor(tmp[:, 0, :], gpart[:, 0, :], gpart[:, 1, :], op=op.add)
                nc.vector.tensor_tensor(partial[:], tmp[:, 0, :], gpart[:, 2, :], op=op.add)

                # ---- squeeze/excite chain for this pair ----
                # sT[c, j] = sum_p partial[p, c] * mask2[p, j]  (mean, transposed)
                sT_ps = psum.tile([P, 4], f32, tag="sT")
                nc.tensor.matmul(sT_ps[:, 0:2], partial[:, 0:P], mask2[:], start=True, stop=True)
                nc.tensor.matmul(sT_ps[:, 2:4], partial[:, P : 2 * P], mask2[:], start=True, stop=True)
                sT_sb = small.tile([P, 4], f32, tag="sTsb")
                nc.scalar.copy(sT_sb[:], sT_ps[:])

                # zT[r, j] = sum_c w1[c, r] * sT[c, j]
                zT_ps = psum.tile([CR, 2], f32, tag="zT")
                nc.tensor.matmul(zT_ps[:], w1s[:, 0, :], sT_sb[:, 0:2], start=True, stop=False)
                nc.tensor.matmul(zT_ps[:], w1s[:, 1, :], sT_sb[:, 2:4], start=False, stop=True)
                zT_sb = small.tile([CR, 2], f32, tag="zTsb")
                nc.vector.tensor_scalar(zT_sb[:], zT_ps[:], 0.0, 6.0, op0=op.max, op1=op.min)

                # y[j, c] = sum_r zT[r, j] * w2[r, c]
                y_ps = psum.tile([2, C], f32, tag="y")
                nc.tensor.matmul(y_ps[:], zT_sb[:], w2s[:], start=True, stop=True)
                g_sb = small.tile([2, C], f32, tag="g")
                nc.vector.tensor_scalar(g_sb[:], y_ps[:], 3.0, 0.0, op0=op.add, op1=op.max)
                nc.vector.tensor_scalar(g_sb[:], g_sb[:], 6.0, 1.0 / 6.0, op0=op.min, op1=op.mult)

                # replicate gate rows onto lines: G[p, c] = g[p // HP, c]
                G_ps = gps.tile([P, C], f32, tag="G")
                nc.tensor.matmul(G_ps[:], rt2[:], g_sb[:], start=True, stop=True)
                G_sb[m] = gpool.tile([P, C], f32, tag="Gsb", name=f"G_sb{m}")
                nc.scalar.copy(G_sb[m][:], G_ps[:])

                # ---- gate multiply in place + stores on the same ring ----
                for (t0, t1) in CHUNKS:
                    xc = X[:, m, t0:t1, :]
                    gb = G_sb[m][:].unsqueeze(1).broadcast_to([P, t1 - t0, C])
                    nc.vector.tensor_tensor(xc, xc, gb, op=op.mult)
                    nc.sync.dma_start(o_d[m, :, :, t0:t1, :], X[:, m, t0:t1, :])

    nc.compile()
    return nc


def _in_maps(x, w1, w2):
    x = np.ascontiguousarray(x, dtype=np.float32)
    w1 = np.ascontiguousarray(w1, dtype=np.float32)
    w2 = np.ascontiguousarray(w2, dtype=np.float32)

    mask2 = np.zeros((P, 2), dtype=np.float32)
    rt2 = np.zeros((2, P), dtype=np.float32)
    for j in range(2):
        mask2[HP * j : HP * (j + 1), j] = 1.0 / TOK
        rt2[j, HP * j : HP * (j + 1)] = 1.0

    in_maps = []
    for c in range(NCORES):
        # [4 samples, 3136 tok, C] -> [NG, 2, HP, TPL, C]
        shard = x[c * BPC : (c + 1) * BPC].reshape(NG, 2, HP, TPL, C)
        in_maps.append({"x": shard, "w1": w1, "w2": w2, "mask2": mask2, "rt2": rt2})
    return in_maps


def kernel(x, w1, w2):
    from concourse.bass_utils import run_bass_kernel_spmd

    if "nc" not in _CACHE:
        _CACHE["nc"] = _build()
    nc = _CACHE["nc"]

    res = run_bass_kernel_spmd(nc, _in_maps(x, w1, w2), core_ids=list(range(NCORES)))
    out = np.empty((B, H, W, C), dtype=np.float32)
    for c in range(NCORES):
        out[c * BPC : (c + 1) * BPC] = res.results[c]["out"].reshape(BPC, H, W, C)
    return out
